# revision 6
# baseline (speedup 1.0000x reference)
"""RWKV WKV recurrence kernel for Trainium2 (8 NeuronCores).

Per core = one batch. Channels on partitions (16 groups of 128), time on
the free dim; the T=2048 recurrence per group is one DVE
tensor_tensor_scan (fp32 internal state, fp16 operands).

Design notes (vs a straightforward fp32 version):
- fp16 on-chip everywhere after exp (numerics model: relmax ~1.6e-3):
  PE transposes stream 2-byte rows, SBUF pressure halves.
- exp(k) and ekv = exp(k)*v run BEFORE the transposes (time-major,
  pair-wide), so PE streams fp16 and the post-transpose PSUM->SBUF step
  is a plain contiguous ACT copy (full 2KB PSUM-bank tiles).
- ekv mul stays on DVE: GpSimd tensor ops contend ~4x on the shared
  DVE/GPSIMD SBUF ports (measured), so Pool only does tiny memsets.
- group-PAIR loads/stores: 1KB DMA descriptors (halves descriptor
  pressure vs 512B).
"""

import os
import sys
from contextlib import ExitStack

import numpy as np

for _p in ("/opt/trn_rl_repo", "/root/.axon_site/_ro/trn_rl_repo"):
    if os.path.isdir(_p) and _p not in sys.path:
        sys.path.insert(0, _p)

import concourse.bacc as bacc
import concourse.mybir as mybir
import concourse.tile as tile
from concourse import dve_ops as _dve_ops
from concourse import dve_spec as _dve_spec
from concourse import masks
from concourse.bass_utils import run_bass_kernel_spmd
from concourse.dve_spec import Spec as _Spec, lower as _dve_lower
from concourse.dve_uop import AluOp as _AluOp, DveOpSpec as _DveOpSpec

F32 = mybir.dt.float32
F16 = mybir.dt.float16
AF = mybir.ActivationFunctionType
OP = mybir.AluOpType

_DIV_C0, _DIV_C1, _DIV_C2 = -0.7071067, -0.1665221, -0.013060556


def _div_mul_ref(in0, in1, c0, c1, c2):
    in0 = np.asarray(in0, np.float32)
    in1 = np.asarray(in1, np.float32)
    n = (~in0.view(np.int32)).view(np.float32)
    s = (in0 * n).astype(np.float32)
    q = (in1 * n).astype(np.float32)
    u = (s * np.float32(c2)).astype(np.float32)
    v = (np.float32(c1) + u).astype(np.float32)
    w = (s * v).astype(np.float32)
    p = (np.float32(c0) + w).astype(np.float32)
    return (q * p).astype(np.float32)


def _register_div_mul():
    name = "WKV_DIV_MUL_ANT"
    if name in _dve_ops._SUB_OPCODE_FOR_NAME:
        return next(o for o in _dve_ops.OPS if o.name == name)
    Src0, Src1 = _dve_spec.Src0, _dve_spec.Src1
    C0, C1, C2 = _dve_spec.C0, _dve_spec.C1, _dve_spec.C2
    _n = _dve_spec.Bin(_AluOp.BITWISE_NOT, Src0, Src0)
    _s = Src0 * _n
    body = (Src1 * _n) * (C0 + _s * (C1 + _s * C2))
    spec = _Spec(body=body, reference=_div_mul_ref)
    shas = {}
    for ver in ("v3", "v4"):
        try:
            uops = _dve_lower(spec, ver=ver)
        except Exception:
            continue
        shas[ver] = _DveOpSpec(name=name, opcode=0, uops=uops, rd1_en=True).sha(ver)
    op = _dve_ops.DveOp(name, spec, subdim=False, uops_sha=shas)
    row = _dve_ops._CUSTOM_DVE_ROW_BASE + len(_dve_ops.OPS)
    assert row < 0x20
    _dve_ops.OPS.append(op)
    _dve_ops._SUB_OPCODE_FOR_NAME[name] = row
    _dve_ops.CUSTOM_DVE_SPECS[name] = spec
    return op


WKV_DIV_MUL = _register_div_mul()

B, T, H = 8, 2048, 2048
N_CORES = 8


def build_nc(t=T, h=H):
    nc = bacc.Bacc("TRN2", target_bir_lowering=False, debug=False)

    key = nc.dram_tensor("key", [t, h], F32, kind="ExternalInput").ap()
    value = nc.dram_tensor("value", [t, h], F32, kind="ExternalInput").ap()
    td = nc.dram_tensor("time_decay", [h], F32, kind="ExternalInput").ap()
    tf = nc.dram_tensor("time_first", [h], F32, kind="ExternalInput").ap()
    out = nc.dram_tensor("out", [t, h], F32, kind="ExternalOutput").ap()

    G = h // 128  # channel groups
    S = t // 128  # time blocks per group
    NB = S // 4  # psum bank tiles (4 blocks each)
    NP = G // 2  # group pairs

    with tile.TileContext(nc) as tc, ExitStack() as ctx:
        const = ctx.enter_context(tc.tile_pool(name="const", bufs=1))
        ident = const.tile([128, 128], F16)
        masks.make_identity(nc, ident[:])

        tf_t = const.tile([128, G], F32)
        nc.sync.dma_start(tf_t[:], tf.rearrange("(g p) -> p g", p=128))
        td_t = const.tile([128, G], F32)
        nc.sync.dma_start(td_t[:], td.rearrange("(g p) -> p g", p=128))
        etd_t = const.tile([128, G], F32)
        nc.scalar.activation(etd_t[:], td_t[:], AF.Exp)
        eu_t = const.tile([128, G], F16)
        nc.scalar.activation(eu_t[:], tf_t[:], AF.Exp)
        ew_t = const.tile([128, G], F16)  # lam = exp(-exp(td))
        nc.scalar.activation(ew_t[:], etd_t[:], AF.Exp, scale=-1.0)

        kin = ctx.enter_context(tc.tile_pool(name="kin", bufs=2))
        tm = ctx.enter_context(tc.tile_pool(name="tm", bufs=1))
        cm = ctx.enter_context(tc.tile_pool(name="cm", bufs=2))
        sc = ctx.enter_context(tc.tile_pool(name="sc", bufs=1))
        nd = ctx.enter_context(tc.tile_pool(name="nd", bufs=1))
        op_pool = ctx.enter_context(tc.tile_pool(name="op", bufs=2))
        ost_pool = ctx.enter_context(tc.tile_pool(name="ost", bufs=2))
        psk = ctx.enter_context(tc.tile_pool(name="psk", bufs=2, space="PSUM"))
        psv = ctx.enter_context(tc.tile_pool(name="psv", bufs=2, space="PSUM"))
        pso = ctx.enter_context(tc.tile_pool(name="pso", bufs=2, space="PSUM"))

        for pg in range(NP):
            h2 = slice(pg * 256, (pg + 1) * 256)

            # ---- pair load (1KB rows), time-major exp / ekv ----
            kc = kin.tile([128, 2 * t], F32, tag="kc")
            nc.sync.dma_start(
                kc[:].rearrange("p (s h) -> p s h", h=256),
                key[:, h2].rearrange("(s p) h -> p s h", p=128),
            )
            vc = kin.tile([128, 2 * t], F32, tag="vc")
            nc.scalar.dma_start(
                vc[:].rearrange("p (s h) -> p s h", h=256),
                value[:, h2].rearrange("(s p) h -> p s h", p=128),
            )
            ekt = tm.tile([128, 2 * t], F16, tag="ekt")
            nc.scalar.activation(ekt[:], kc[:], AF.Exp)
            evt = tm.tile([128, 2 * t], F16, tag="evt")
            nc.vector.tensor_mul(evt[:], ekt[:], vc[:])

            ost = ost_pool.tile([128, 2 * t], F32, tag="ost")
            ost_s = ost[:].rearrange("p (s h) -> p s h", h=256)

            for g2 in range(2):
                g = 2 * pg + g2
                eu_g = eu_t[:, g : g + 1]
                ew_g = ew_t[:, g : g + 1]
                sfx = str(g2)

                # ---- transpose (fp16) + contiguous PSUM->SBUF copies ----
                ekkv = cm.tile([128, 2 * t], F16, tag="ekkv" + sfx)
                ekv = ekkv[:, 0:t]
                ek = ekkv[:, t : 2 * t]
                for nb in range(2):
                    kT = psk.tile([128, 1024], F16, tag="kT")
                    vT = psv.tile([128, 1024], F16, tag="vT")
                    for c8 in range(8):
                        s = nb * 8 + c8
                        bs = slice(s * 256 + g2 * 128, s * 256 + (g2 + 1) * 128)
                        nc.tensor.transpose(
                            kT[:, c8 * 128 : (c8 + 1) * 128], ekt[:, bs], ident[:]
                        )
                        nc.tensor.transpose(
                            vT[:, c8 * 128 : (c8 + 1) * 128], evt[:, bs], ident[:]
                        )
                    bsl = slice(nb * 1024, (nb + 1) * 1024)
                    nc.scalar.copy(ek[:, bsl], kT[:])
                    nc.scalar.copy(ekv[:, bsl], vT[:])

                # ---- the recurrences: one scan per tensor, shared tile ----
                AB = sc.tile([128, 2 * t + 1], F16, tag="AB" + sfx)
                nc.gpsimd.memset(AB[:, 0:1], 0.0)
                nc.vector.tensor_tensor_scan(
                    AB[:, 1 : t + 1],
                    ew_g.broadcast_to((128, t)),
                    ekv,
                    0.0,
                    OP.mult,
                    OP.add,
                )
                nc.gpsimd.memset(AB[:, t : t + 1], 0.0)
                nc.vector.tensor_tensor_scan(
                    AB[:, t + 1 : 2 * t + 1],
                    ew_g.broadcast_to((128, t)),
                    ek,
                    0.0,
                    OP.mult,
                    OP.add,
                )

                numden = nd.tile([128, 2 * t], F16, tag="numden" + sfx)
                nc.vector.scalar_tensor_tensor(
                    numden[:], ekkv[:], eu_g, AB[:, 0 : 2 * t], OP.mult, OP.add
                )
                outp = op_pool.tile([128, t], F16, tag="outp" + sfx)
                nc.vector._custom_dve(
                    WKV_DIV_MUL,
                    out=outp[:],
                    in0=numden[:, t : 2 * t],
                    in1=numden[:, 0:t],
                    s0=_DIV_C0,
                    s1=_DIV_C1,
                    imm2=_DIV_C2,
                )

                # ---- transpose back + cast fp32 into pair staging ----
                for nb in range(2):
                    oT = pso.tile([128, 1024], F16, tag="oT")
                    for c8 in range(8):
                        s = nb * 8 + c8
                        nc.tensor.transpose(
                            oT[:, c8 * 128 : (c8 + 1) * 128],
                            outp[:, s * 128 : (s + 1) * 128],
                            ident[:],
                        )
                    nc.scalar.copy(
                        ost_s[:, nb * 8 : nb * 8 + 8, g2 * 128 : (g2 + 1) * 128],
                        oT[:].rearrange("p (s h) -> p s h", h=128),
                    )

            nc.sync.dma_start(
                out[:, h2].rearrange("(s p) h -> p s h", p=128),
                ost[:].rearrange("p (s h) -> p s h", h=256),
            )

    nc.compile()
    return nc


_nc_cache = {}


def _get_nc():
    if "nc" not in _nc_cache:
        _nc_cache["nc"] = build_nc()
    return _nc_cache["nc"]


def kernel_with_results(key, value, time_decay, time_first, trace=False, tmpdir=None):
    nc = _get_nc()
    key = np.ascontiguousarray(key, dtype=np.float32)
    value = np.ascontiguousarray(value, dtype=np.float32)
    time_decay = np.ascontiguousarray(time_decay, dtype=np.float32)
    time_first = np.ascontiguousarray(time_first, dtype=np.float32)
    in_maps = [
        {
            "key": key[i],
            "value": value[i],
            "time_decay": time_decay,
            "time_first": time_first,
        }
        for i in range(N_CORES)
    ]
    res = run_bass_kernel_spmd(
        nc, in_maps, list(range(N_CORES)), trace=trace, tmpdir=tmpdir
    )
    out = np.stack([res.results[i]["out"] for i in range(N_CORES)], axis=0)
    return out, res


def kernel(key, value, time_decay, time_first):
    out, _ = kernel_with_results(key, value, time_decay, time_first)
    return out
